# revision 24
# baseline (speedup 1.0000x reference)
"""Multi-head attention (N=4, T=2048, D=512, H=8, dh=64) on 8 TRN2 NeuronCores.

Sharding: batch N (4) x head-group (2 groups of 4 heads) -> 8 cores.
Each core computes, for its (batch n, head-group g):
  q = query[n] @ Wq[:, 256g:256g+256]   (as qT, [256, 2048])
  k = key[n]   @ Wk[:, ...]             (as kT)
  v = key[n]   @ Wv[:, ...]             (as V tiles [t, dh] with ones column)
  per head h' in 0..3, per q-block of 512:
    ST[k, q] = K-tile matmuls (contraction dh=64, bf16)
    P = exp(ST / sqrt(512))  (ScalarE, multi-bank PSUM read)
    OT[65, 512] += [V | 1]^T @ P  (row 64 = softmax denominators)
    out = OT[0:64] * broadcast(1 / OT[64])
Host reassembles out[n, :, 256g:256g+256] = oT.T.

The kernel is ScalarE(exp)-bound (~127us of Exp activations; fp8 matmuls
were tried and fail the accuracy budget, so everything is bf16 and PE is
~130us, co-critical). The schedule keeps ScalarE busy back-to-back:
  - startup: only K-proj + Q-proj(qb0) gate the first exp; staging DMAs
    are split across the SP and Activation HWDGE queues.
  - all other projections (K dt2=1, V per-head, Q blocks) are emitted in
    bounded chunks at pair ends, sized to the PE slack in each pair's exp
    window (pair 0 has no O-accumulation yet, so it takes the big chunk).
  - the attention loop is software-pipelined: pair p's score/exp phase
    interleaves pair p-1's O-accumulation + normalization; the last pair
    O-accumulates eagerly after each exp group to shorten the tail.
  - projections borrow PSUM transiently: startup/deferred ones cycle
    through the C ring between O-tile lifetimes (emission order keeps the
    ring free of deadlocks).
"""

import math

import ml_dtypes
import numpy as np

import concourse.bass as bass
import concourse.mybir as mybir
import concourse.tile as tile
from concourse import bacc
from concourse.bass_utils import run_bass_kernel_spmd

F32 = mybir.dt.float32
BF16 = mybir.dt.bfloat16
EXP = mybir.ActivationFunctionType.Exp

N, T, D = 4, 2048, 512
HPC, DH = 4, 64          # heads per core, head dim
GC = HPC * DH            # head-group columns (256)
SCALE = 1.0 / math.sqrt(D)
QB = 512                 # q block
NQB = T // QB            # 4
NKT = T // 128           # 16 k tiles
KS = D // 128            # 4 contraction slices for projections

# exp-group pattern per (head, qblock): (pool_key, n_ktiles). Pools A (4 banks)
# and B (2 banks) alternate so TensorE score matmuls overlap ScalarE exp.
GROUPS = (("A", 2), ("B", 2), ("A", 4), ("B", 2), ("A", 4), ("B", 2))


def build():
    nc = bacc.Bacc("TRN2", target_bir_lowering=False, debug=False, num_devices=8)
    qT_in = nc.declare_dram_parameter("qT", [D, T], BF16, isOutput=False)
    kT_in = nc.declare_dram_parameter("kT", [D, T], BF16, isOutput=False)
    wq_in = nc.declare_dram_parameter("wq", [D, GC], BF16, isOutput=False)
    wk_in = nc.declare_dram_parameter("wk", [D, GC], BF16, isOutput=False)
    wv_in = nc.declare_dram_parameter("wv", [D, GC], BF16, isOutput=False)
    oT_out = nc.declare_dram_parameter("oT", [GC, T], F32, isOutput=True)

    with tile.TileContext(nc) as tc:
        with (
            tc.tile_pool(name="stage", bufs=1) as stage,
            tc.tile_pool(name="const", bufs=1) as const,
            tc.tile_pool(name="act", bufs=1) as actp,
            tc.tile_pool(name="pt", bufs=3) as ptp,
            tc.tile_pool(name="small", bufs=4) as small,
            tc.tile_pool(name="psA", bufs=1, space="PSUM") as psA,
            tc.tile_pool(name="psB", bufs=1, space="PSUM") as psB,
            tc.tile_pool(name="psC", bufs=2, space="PSUM") as psC,
        ):
            # ---- input staging on the two HWDGE queues (SP + Activation).
            # gpsimd SWDGE costs ~3.2us of sequencer per strided DMA: not used.
            # Critical path: kin (K-proj) and qin block 0 + wq (Q-proj qb0).
            ws = {}
            for nm in ("wk", "wv", "wq"):
                ws[nm] = const.tile([128, KS, GC], BF16, tag=nm, name=nm)
            kin = stage.tile([128, KS, T], BF16, tag="kin")
            qin = stage.tile([128, KS, T], BF16, tag="qin")

            def dma_in(eng, dst, src_ap):
                eng.dma_start(dst, src_ap)

            kT_r = kT_in.rearrange("(s p) t -> p s t", p=128)
            qT_r = qT_in.rearrange("(s p) t -> p s t", p=128)
            wk_r = wk_in.rearrange("(s p) c -> p s c", p=128)
            wq_r = wq_in.rearrange("(s p) c -> p s c", p=128)
            wv_r = wv_in.rearrange("(s p) c -> p s c", p=128)

            def kin_tb(tb):
                return (kin[:, :, tb * QB : (tb + 1) * QB],
                        kT_r[:, :, tb * QB : (tb + 1) * QB])

            def qin_tb(tb):
                return (qin[:, :, tb * QB : (tb + 1) * QB],
                        qT_r[:, :, tb * QB : (tb + 1) * QB])

            # one serial ~360GB/s DMA pipe: global order matters, queues only
            # pipeline the config step. Order: wk, wq, kin0, qin0, wv, kin1-3,
            # then the remaining q blocks.
            nc.sync.dma_start(ws["wk"][:], wk_r)
            nc.scalar.dma_start(ws["wq"][:], wq_r)
            nc.sync.dma_start(*qin_tb(0))
            nc.scalar.dma_start(*kin_tb(0))
            nc.sync.dma_start(*kin_tb(1))
            nc.scalar.dma_start(*kin_tb(2))
            nc.sync.dma_start(*kin_tb(3))
            nc.scalar.dma_start(ws["wv"][:], wv_r)
            nc.sync.dma_start(*qin_tb(1))
            nc.scalar.dma_start(*qin_tb(2))
            nc.sync.dma_start(*qin_tb(3))

            kT_att = [
                actp.tile([128, T], BF16, tag=f"ka{d}", name=f"ka{d}")
                for d in range(2)
            ]
            qT_att = [
                actp.tile([128, T], BF16, tag=f"qa{d}", name=f"qa{d}")
                for d in range(2)
            ]

            def emit_kproj_tb(dt2, tb, copy_eng="dve"):
                ps = psC.tile([128, QB], F32, tag="C", name="kproj_ps")
                for s in range(KS):
                    nc.tensor.matmul(
                        ps[:],
                        ws["wk"][:, s, dt2 * 128 : (dt2 + 1) * 128],
                        kin[:, s, tb * QB : (tb + 1) * QB],
                        start=(s == 0),
                        stop=(s == KS - 1),
                    )
                dst = kT_att[dt2][:, tb * QB : (tb + 1) * QB]
                if copy_eng == "act":
                    # ScalarE is idle pre-attention: Copy activation offloads
                    # the startup-critical PSUM->SBUF move from DVE
                    nc.scalar.copy(dst, ps[:])
                else:
                    nc.vector.tensor_copy(dst, ps[:])

            def emit_qproj(qb, dt2, copy_eng="dve"):
                ps = psC.tile([128, QB], F32, tag="C", name="qproj_ps")
                for s in range(KS):
                    nc.tensor.matmul(
                        ps[:],
                        ws["wq"][:, s, dt2 * 128 : (dt2 + 1) * 128],
                        qin[:, s, qb * QB : (qb + 1) * QB],
                        start=(s == 0),
                        stop=(s == KS - 1),
                    )
                dst = qT_att[dt2][:, qb * QB : (qb + 1) * QB]
                if copy_eng == "act":
                    nc.scalar.copy(dst, ps[:])
                else:
                    nc.vector.tensor_copy(dst, ps[:])

            # ---- V tiles [128, kt, head, 65] with ones column ----
            vp = const.tile([128, NKT, HPC, DH + 1], BF16, tag="vp")
            ones_f32 = const.tile([128, NKT * HPC], F32, tag="ones")
            nc.vector.memset(ones_f32[:], 1.0)
            nc.vector.tensor_copy(
                vp[:, :, :, DH : DH + 1],
                ones_f32[:].rearrange("p (a b) -> p a b", b=HPC).unsqueeze(3),
            )

            def emit_vproj(tt):
                # all 4 heads' V for one k-tile: out [128 kpos, 256]
                ps = psC.tile([128, QB], F32, tag="C", name="vproj_ps")
                for s in range(KS):
                    nc.tensor.matmul(
                        ps[:, 0:GC],
                        kin[:, s, tt * 128 : (tt + 1) * 128],
                        ws["wv"][:, s, :],
                        start=(s == 0),
                        stop=(s == KS - 1),
                    )
                nc.vector.tensor_copy(
                    vp[:, tt, :, 0:DH],
                    ps[:, 0:GC].rearrange("p (h d) -> p h d", d=DH),
                )

            # ---- attention pieces ----
            def emit_s_group(qb, hp, pt, gi):
                pool_key, nkt = GROUPS[gi]
                kt0 = sum(n for _, n in GROUPS[:gi])
                tile2, base = hp // 2, DH * (hp % 2)
                q_src = qT_att[tile2][base : base + DH, qb * QB : (qb + 1) * QB]
                pool = psA if pool_key == "A" else psB
                width = 2048 if pool_key == "A" else 1024
                ps = pool.tile([128, width], F32, tag=pool_key, name="s_ps")
                for l in range(nkt):
                    kt = kt0 + l
                    nc.tensor.matmul(
                        ps[:, l * QB : (l + 1) * QB],
                        kT_att[tile2][base : base + DH, kt * 128 : (kt + 1) * 128],
                        q_src,
                        start=True,
                        stop=True,
                    )
                nc.scalar.activation(
                    pt[:, kt0 * QB : (kt0 + nkt) * QB],
                    ps[:, : nkt * QB],
                    EXP,
                    scale=SCALE,
                )

            def emit_o_chunk(st, kt_lo, kt_hi):
                qb, hp, pt, po = st
                for kt in range(kt_lo, kt_hi):
                    nc.tensor.matmul(
                        po[0 : DH + 1],
                        vp[:, kt, hp, :],
                        pt[:, kt * QB : (kt + 1) * QB],
                        start=(kt == 0),
                        stop=(kt == NKT - 1),
                    )

            def emit_norm(st):
                qb, hp, pt, po = st
                sums = small.tile([1, QB], F32, tag="sums", name="sums")
                nc.vector.tensor_copy(sums[:], po[DH : DH + 1, :])
                rec = small.tile([1, QB], F32, tag="rec", name="rec")
                nc.vector.reciprocal_approx_fast(rec[:], sums[:])
                bc = small.tile([DH, QB], F32, tag="bc", name="bc")
                nc.gpsimd.partition_broadcast(bc[:], rec[:])
                ot = small.tile([DH, QB], F32, tag="ot", name="ot")
                nc.vector.tensor_mul(ot[:], po[0:DH, :], bc[:])
                nc.sync.dma_start(
                    oT_out[hp * DH : (hp + 1) * DH, qb * QB : (qb + 1) * QB],
                    ot[:],
                )

            # ---- schedule ----
            # Pair order is head-pair-major: all q blocks for heads 0-1 first,
            # then heads 2-3 -- this pushes the K-proj dt2=1 deadline from
            # pair 2 out to pair 8 so it amortizes into exp windows.
            #
            # AV(p) is straddled: chunk1 (kt 0-8) at the end of pair p+1,
            # chunk2 (kt 8-16) + norm after group 3 of pair p+2. That caps
            # per-slot PE bursts at ~1.7us, leaving slot room for deferred
            # projections (V-proj, K dt2=1, Q blocks) without stalling exp.
            pairs = [
                (qb, hp)
                for dt2 in (0, 1)
                for qb in range(NQB)
                for hp in (2 * dt2, 2 * dt2 + 1)
            ]
            last = len(pairs) - 1
            st = {}

            def new_po(pi):
                po = psC.tile([128, QB], F32, tag="C", name="po")
                st[pi] = (*st[pi], po)
                return st[pi]

            g1_slot = {
                1: [lambda: [emit_vproj(tt) for tt in range(8, 12)]],
                2: [lambda: [emit_vproj(tt) for tt in range(12, 16)]],
                4: [lambda: emit_kproj_tb(1, 0)],
                5: [lambda: emit_kproj_tb(1, 1)],
                6: [lambda: emit_kproj_tb(1, 2)],
                7: [lambda: emit_kproj_tb(1, 3)],
            }
            g3_extra = {}
            end_extra = {
                1: [lambda: emit_qproj(1, 0)],
                3: [lambda: emit_qproj(2, 0)],
                5: [lambda: emit_qproj(3, 0)],
                6: [lambda: emit_qproj(0, 1)],
                8: [lambda: emit_qproj(1, 1)],
                10: [lambda: emit_qproj(2, 1)],
                12: [lambda: emit_qproj(3, 1)],
            }

            # PE warmup: ~12 junk matmuls with no data deps ride out the
            # p-state ramp while the first DMAs are in flight. They write the
            # first psA ring slot, which pair 0's group 0 then overwrites
            # (start=True resets the accumulation region).
            dummy = const.tile([64, 128], BF16, tag="dummy")
            nc.gpsimd.memset(dummy[:], 0.0)
            wps = psA.tile([128, 2048], F32, tag="A", name="warm_ps")
            for _ in range(32):
                nc.tensor.matmul(
                    wps[:, 0:128],
                    dummy[:],
                    dummy[:],
                    start=True,
                    stop=True,
                )

            # startup: Q-proj(qb0) first (its q block is first on the wire),
            # then K-proj dt2=0 per arriving kin block, interleaved with
            # pair 0's score groups below.
            emit_qproj(0, 0, copy_eng="act")
            emit_kproj_tb(0, 0, copy_eng="act")

            for pi, (qb, hp) in enumerate(pairs):
                pt = ptp.tile([128, NKT * QB], BF16, tag="pt", name="pt")
                st[pi] = (qb, hp, pt)
                emit_s_group(qb, hp, pt, 0)
                emit_s_group(qb, hp, pt, 1)
                # g1 slot
                for fn in g1_slot.get(pi, ()):
                    fn()
                if pi == 0:
                    emit_kproj_tb(0, 1)
                if pi == last:
                    emit_o_chunk(new_po(pi - 1), 0, 8)
                emit_s_group(qb, hp, pt, 2)
                if pi == 0:
                    emit_kproj_tb(0, 2)
                emit_s_group(qb, hp, pt, 3)
                # g3 slot
                if pi == 0:
                    emit_kproj_tb(0, 3)
                straddle = pi >= 2 and pi - 2 in st and len(st[pi - 2]) == 4
                if straddle:
                    emit_o_chunk(st[pi - 2], 8, 12)
                for fn in g3_extra.get(pi, ()):
                    fn()
                if pi == last:
                    emit_o_chunk(st[pi - 1], 8, NKT)
                    emit_norm(st[pi - 1])
                    cur = new_po(pi)
                    emit_o_chunk(cur, 0, 4)
                    emit_o_chunk(cur, 4, 10)
                emit_s_group(qb, hp, pt, 4)
                if straddle:
                    emit_o_chunk(st[pi - 2], 12, NKT)
                    emit_norm(st[pi - 2])
                emit_s_group(qb, hp, pt, 5)
                # end slot
                if pi == 0:
                    for tt in range(8):
                        emit_vproj(tt)
                if 1 <= pi < last:
                    emit_o_chunk(new_po(pi - 1), 0, 8)
                for fn in end_extra.get(pi, ()):
                    fn()
                if pi == last:
                    emit_o_chunk(st[pi], 10, NKT)
                    emit_norm(st[pi])

    nc.compile()
    return nc


_NC = None


def _get_nc():
    global _NC
    if _NC is None:
        _NC = build()
    return _NC


def run(query, key, W_query, W_key, W_value, trace=False):
    nc = _get_nc()
    query = np.asarray(query, dtype=np.float32)
    key = np.asarray(key, dtype=np.float32)
    W_query = np.asarray(W_query, dtype=np.float32)
    W_key = np.asarray(W_key, dtype=np.float32)
    W_value = np.asarray(W_value, dtype=np.float32)

    bf = ml_dtypes.bfloat16
    in_maps = []
    for c in range(8):
        n, g = c // 2, c % 2
        cols = slice(g * GC, (g + 1) * GC)
        in_maps.append(
            {
                "qT": np.ascontiguousarray(query[n].T.astype(bf)),
                "kT": np.ascontiguousarray(key[n].T.astype(bf)),
                "wq": np.ascontiguousarray(W_query[:, cols].astype(bf)),
                "wk": np.ascontiguousarray(W_key[:, cols].astype(bf)),
                "wv": np.ascontiguousarray(W_value[:, cols].astype(bf)),
            }
        )
    res = run_bass_kernel_spmd(nc, in_maps, core_ids=list(range(8)), trace=trace)
    out = np.empty((N, T, D), dtype=np.float32)
    for c in range(8):
        n, g = c // 2, c % 2
        out[n, :, g * GC : (g + 1) * GC] = res.results[c]["oT"].T
    return out, res


def kernel(query, key, W_query, W_key, W_value):
    out, _ = run(query, key, W_query, W_key, W_value, trace=False)
    return out


# revision 29
# speedup vs baseline: 1.0065x; 1.0065x over previous
"""Multi-head attention (N=4, T=2048, D=512, H=8, dh=64) on 8 TRN2 NeuronCores.

Sharding: batch N (4) x head-group (2 groups of 4 heads) -> 8 cores.
Each core computes, for its (batch n, head-group g):
  q = query[n] @ Wq[:, 256g:256g+256]   (as qT, [256, 2048])
  k = key[n]   @ Wk[:, ...]             (as kT)
  v = key[n]   @ Wv[:, ...]             (as V tiles [t, dh] with ones column)
  per head h' in 0..3, per q-block of 512:
    ST[k, q] = K-tile matmuls (contraction dh=64, bf16)
    P = exp(ST / sqrt(512))  (ScalarE, multi-bank PSUM read)
    OT[65, 512] += [V | 1]^T @ P  (row 64 = softmax denominators)
    out = OT[0:64] * broadcast(1 / OT[64])
Host reassembles out[n, :, 256g:256g+256] = oT.T.

The kernel is ScalarE(exp)-bound (~127us of Exp activations; fp8 matmuls
were tried and fail the accuracy budget, so everything is bf16 and PE is
~130us, co-critical). The schedule keeps ScalarE busy back-to-back:
  - startup: only K-proj + Q-proj(qb0) gate the first exp; staging DMAs
    are split across the SP and Activation HWDGE queues.
  - all other projections (K dt2=1, V per-head, Q blocks) are emitted in
    bounded chunks at pair ends, sized to the PE slack in each pair's exp
    window (pair 0 has no O-accumulation yet, so it takes the big chunk).
  - the attention loop is software-pipelined: pair p's score/exp phase
    interleaves pair p-1's O-accumulation + normalization; the last pair
    O-accumulates eagerly after each exp group to shorten the tail.
  - projections borrow PSUM transiently: startup/deferred ones cycle
    through the C ring between O-tile lifetimes (emission order keeps the
    ring free of deadlocks).
"""

import math

import ml_dtypes
import numpy as np

import concourse.bass as bass
import concourse.mybir as mybir
import concourse.tile as tile
from concourse import bacc
from concourse.bass_utils import run_bass_kernel_spmd

F32 = mybir.dt.float32
BF16 = mybir.dt.bfloat16
EXP = mybir.ActivationFunctionType.Exp

N, T, D = 4, 2048, 512
HPC, DH = 4, 64          # heads per core, head dim
GC = HPC * DH            # head-group columns (256)
SCALE = 1.0 / math.sqrt(D)
QB = 512                 # q block
NQB = T // QB            # 4
NKT = T // 128           # 16 k tiles
KS = D // 128            # 4 contraction slices for projections

# exp-group pattern per (head, qblock): (pool_key, n_ktiles). Pools A (4 banks)
# and B (2 banks) alternate so TensorE score matmuls overlap ScalarE exp.
GROUPS = (("A", 2), ("B", 2), ("A", 4), ("B", 2), ("A", 4), ("B", 2))


def build():
    nc = bacc.Bacc("TRN2", target_bir_lowering=False, debug=False, num_devices=8)
    qT_in = nc.declare_dram_parameter("qT", [D, T], BF16, isOutput=False)
    kT_in = nc.declare_dram_parameter("kT", [D, T], BF16, isOutput=False)
    wq_in = nc.declare_dram_parameter("wq", [D, GC], BF16, isOutput=False)
    wk_in = nc.declare_dram_parameter("wk", [D, GC], BF16, isOutput=False)
    wv_in = nc.declare_dram_parameter("wv", [D, GC], BF16, isOutput=False)
    oT_out = nc.declare_dram_parameter("oT", [GC, T], F32, isOutput=True)

    with tile.TileContext(nc) as tc:
        with (
            tc.tile_pool(name="stage", bufs=1) as stage,
            tc.tile_pool(name="const", bufs=1) as const,
            tc.tile_pool(name="act", bufs=1) as actp,
            tc.tile_pool(name="pt", bufs=3) as ptp,
            tc.tile_pool(name="small", bufs=4) as small,
            tc.tile_pool(name="psA", bufs=1, space="PSUM") as psA,
            tc.tile_pool(name="psB", bufs=1, space="PSUM") as psB,
            tc.tile_pool(name="psC", bufs=2, space="PSUM") as psC,
        ):
            # ---- input staging on the two HWDGE queues (SP + Activation).
            # gpsimd SWDGE costs ~3.2us of sequencer per strided DMA: not used.
            # Critical path: kin (K-proj) and qin block 0 + wq (Q-proj qb0).
            ws = {}
            for nm in ("wk", "wv", "wq"):
                ws[nm] = const.tile([128, KS, GC], BF16, tag=nm, name=nm)
            kin = stage.tile([128, KS, T], BF16, tag="kin")
            qin = stage.tile([128, KS, T], BF16, tag="qin")

            def dma_in(eng, dst, src_ap):
                eng.dma_start(dst, src_ap)

            kT_r = kT_in.rearrange("(s p) t -> p s t", p=128)
            qT_r = qT_in.rearrange("(s p) t -> p s t", p=128)
            wk_r = wk_in.rearrange("(s p) c -> p s c", p=128)
            wq_r = wq_in.rearrange("(s p) c -> p s c", p=128)
            wv_r = wv_in.rearrange("(s p) c -> p s c", p=128)

            def kin_tb(tb):
                return (kin[:, :, tb * QB : (tb + 1) * QB],
                        kT_r[:, :, tb * QB : (tb + 1) * QB])

            def qin_tb(tb):
                return (qin[:, :, tb * QB : (tb + 1) * QB],
                        qT_r[:, :, tb * QB : (tb + 1) * QB])

            # one serial ~360GB/s DMA pipe: global order matters, queues only
            # pipeline the config step. Order: wk, wq, kin0, qin0, wv, kin1-3,
            # then the remaining q blocks.
            nc.sync.dma_start(ws["wk"][:], wk_r)
            nc.scalar.dma_start(ws["wq"][:], wq_r)
            nc.sync.dma_start(*qin_tb(0))
            nc.scalar.dma_start(*kin_tb(0))
            nc.sync.dma_start(*kin_tb(1))
            nc.scalar.dma_start(*kin_tb(2))
            nc.sync.dma_start(*kin_tb(3))
            nc.scalar.dma_start(ws["wv"][:], wv_r)
            nc.sync.dma_start(*qin_tb(1))
            nc.scalar.dma_start(*qin_tb(2))
            nc.sync.dma_start(*qin_tb(3))

            kT_att = [
                actp.tile([128, T], BF16, tag=f"ka{d}", name=f"ka{d}")
                for d in range(2)
            ]
            qT_att = [
                actp.tile([128, T], BF16, tag=f"qa{d}", name=f"qa{d}")
                for d in range(2)
            ]

            def emit_kproj_tb(dt2, tb, copy_eng="dve"):
                ps = psC.tile([128, QB], F32, tag="C", name="kproj_ps")
                for s in range(KS):
                    nc.tensor.matmul(
                        ps[:],
                        ws["wk"][:, s, dt2 * 128 : (dt2 + 1) * 128],
                        kin[:, s, tb * QB : (tb + 1) * QB],
                        start=(s == 0),
                        stop=(s == KS - 1),
                    )
                dst = kT_att[dt2][:, tb * QB : (tb + 1) * QB]
                if copy_eng == "act":
                    # ScalarE is idle pre-attention: Copy activation offloads
                    # the startup-critical PSUM->SBUF move from DVE
                    nc.scalar.copy(dst, ps[:])
                else:
                    nc.vector.tensor_copy(dst, ps[:])

            def emit_qproj(qb, dt2, copy_eng="dve"):
                ps = psC.tile([128, QB], F32, tag="C", name="qproj_ps")
                for s in range(KS):
                    nc.tensor.matmul(
                        ps[:],
                        ws["wq"][:, s, dt2 * 128 : (dt2 + 1) * 128],
                        qin[:, s, qb * QB : (qb + 1) * QB],
                        start=(s == 0),
                        stop=(s == KS - 1),
                    )
                dst = qT_att[dt2][:, qb * QB : (qb + 1) * QB]
                if copy_eng == "act":
                    nc.scalar.copy(dst, ps[:])
                else:
                    nc.vector.tensor_copy(dst, ps[:])

            # ---- V tiles [128, kt, head, 65] with ones column ----
            vp = const.tile([128, NKT, HPC, DH + 1], BF16, tag="vp")
            ones_f32 = const.tile([128, NKT * HPC], F32, tag="ones")
            nc.vector.memset(ones_f32[:], 1.0)
            nc.vector.tensor_copy(
                vp[:, :, :, DH : DH + 1],
                ones_f32[:].rearrange("p (a b) -> p a b", b=HPC).unsqueeze(3),
            )

            def emit_vproj(tt):
                # all 4 heads' V for one k-tile: out [128 kpos, 256]
                ps = psC.tile([128, QB], F32, tag="C", name="vproj_ps")
                for s in range(KS):
                    nc.tensor.matmul(
                        ps[:, 0:GC],
                        kin[:, s, tt * 128 : (tt + 1) * 128],
                        ws["wv"][:, s, :],
                        start=(s == 0),
                        stop=(s == KS - 1),
                    )
                nc.vector.tensor_copy(
                    vp[:, tt, :, 0:DH],
                    ps[:, 0:GC].rearrange("p (h d) -> p h d", d=DH),
                )

            # ---- attention pieces ----
            def emit_s_group(qb, hp, pt, gi):
                pool_key, nkt = GROUPS[gi]
                kt0 = sum(n for _, n in GROUPS[:gi])
                tile2, base = hp // 2, DH * (hp % 2)
                q_src = qT_att[tile2][base : base + DH, qb * QB : (qb + 1) * QB]
                pool = psA if pool_key == "A" else psB
                width = 2048 if pool_key == "A" else 1024
                ps = pool.tile([128, width], F32, tag=pool_key, name="s_ps")
                for l in range(nkt):
                    kt = kt0 + l
                    nc.tensor.matmul(
                        ps[:, l * QB : (l + 1) * QB],
                        kT_att[tile2][base : base + DH, kt * 128 : (kt + 1) * 128],
                        q_src,
                        start=True,
                        stop=True,
                    )
                nc.scalar.activation(
                    pt[:, kt0 * QB : (kt0 + nkt) * QB],
                    ps[:, : nkt * QB],
                    EXP,
                    scale=SCALE,
                )

            def emit_o_chunk(st, kt_lo, kt_hi):
                qb, hp, pt, po = st
                for kt in range(kt_lo, kt_hi):
                    nc.tensor.matmul(
                        po[0 : DH + 1],
                        vp[:, kt, hp, :],
                        pt[:, kt * QB : (kt + 1) * QB],
                        start=(kt == 0),
                        stop=(kt == NKT - 1),
                    )

            def emit_norm(st, halves=1):
                # halves=2 pipelines the copy/recip/broadcast/mul/dma chain
                # across DVE/Pool/SP for the tail-critical last pair
                qb, hp, pt, po = st
                hw = QB // halves
                for h in range(halves):
                    cs = slice(h * hw, (h + 1) * hw)
                    sums = small.tile([1, hw], F32, tag="sums", name="sums")
                    nc.vector.tensor_copy(sums[:], po[DH : DH + 1, cs])
                    rec = small.tile([1, hw], F32, tag="rec", name="rec")
                    nc.vector.reciprocal_approx_fast(rec[:], sums[:])
                    bc = small.tile([DH, hw], F32, tag="bc", name="bc")
                    nc.gpsimd.partition_broadcast(bc[:], rec[:])
                    ot = small.tile([DH, hw], F32, tag="ot", name="ot")
                    nc.vector.tensor_mul(ot[:], po[0:DH, cs], bc[:])
                    nc.sync.dma_start(
                        oT_out[
                            hp * DH : (hp + 1) * DH,
                            qb * QB + h * hw : qb * QB + (h + 1) * hw,
                        ],
                        ot[:],
                    )

            # ---- schedule ----
            # Pair order is head-pair-major: all q blocks for heads 0-1 first,
            # then heads 2-3 -- this pushes the K-proj dt2=1 deadline from
            # pair 2 out to pair 8 so it amortizes into exp windows.
            #
            # AV(p) is straddled: chunk1 (kt 0-8) at the end of pair p+1,
            # chunk2 (kt 8-16) + norm after group 3 of pair p+2. That caps
            # per-slot PE bursts at ~1.7us, leaving slot room for deferred
            # projections (V-proj, K dt2=1, Q blocks) without stalling exp.
            pairs = [
                (qb, hp)
                for dt2 in (0, 1)
                for qb in range(NQB)
                for hp in (2 * dt2, 2 * dt2 + 1)
            ]
            last = len(pairs) - 1
            st = {}

            def new_po(pi):
                po = psC.tile([128, QB], F32, tag="C", name="po")
                st[pi] = (*st[pi], po)
                return st[pi]

            g1_slot = {
                1: [lambda: [emit_vproj(tt) for tt in range(8, 12)]],
                2: [lambda: [emit_vproj(tt) for tt in range(12, 16)]],
                4: [lambda: emit_kproj_tb(1, 0)],
                5: [lambda: emit_kproj_tb(1, 1)],
                6: [lambda: emit_kproj_tb(1, 2)],
                7: [lambda: emit_kproj_tb(1, 3)],
            }
            g3_extra = {}
            end_extra = {
                1: [lambda: emit_qproj(1, 0)],
                3: [lambda: emit_qproj(2, 0)],
                5: [lambda: emit_qproj(3, 0)],
                6: [lambda: emit_qproj(0, 1)],
                8: [lambda: emit_qproj(1, 1)],
                10: [lambda: emit_qproj(2, 1)],
                12: [lambda: emit_qproj(3, 1)],
            }

            # PE warmup: ~12 junk matmuls with no data deps ride out the
            # p-state ramp while the first DMAs are in flight. They write the
            # first psA ring slot, which pair 0's group 0 then overwrites
            # (start=True resets the accumulation region).
            dummy = const.tile([64, 128], BF16, tag="dummy")
            nc.gpsimd.memset(dummy[:], 0.0)
            wps = psA.tile([128, 2048], F32, tag="A", name="warm_ps")
            for _ in range(32):
                nc.tensor.matmul(
                    wps[:, 0:128],
                    dummy[:],
                    dummy[:],
                    start=True,
                    stop=True,
                )

            # startup: Q-proj(qb0) first (its q block is first on the wire),
            # then K-proj dt2=0 per arriving kin block, interleaved with
            # pair 0's score groups below.
            emit_qproj(0, 0, copy_eng="act")
            emit_kproj_tb(0, 0, copy_eng="act")

            for pi, (qb, hp) in enumerate(pairs):
                pt = ptp.tile([128, NKT * QB], BF16, tag="pt", name="pt")
                st[pi] = (qb, hp, pt)
                emit_s_group(qb, hp, pt, 0)
                emit_s_group(qb, hp, pt, 1)
                # g1 slot
                for fn in g1_slot.get(pi, ()):
                    fn()
                if pi == 0:
                    emit_kproj_tb(0, 1)
                if pi == last:
                    emit_o_chunk(new_po(pi - 1), 0, 8)
                emit_s_group(qb, hp, pt, 2)
                if pi == 0:
                    emit_kproj_tb(0, 2)
                emit_s_group(qb, hp, pt, 3)
                # g3 slot
                if pi == 0:
                    emit_kproj_tb(0, 3)
                straddle = pi >= 2 and pi - 2 in st and len(st[pi - 2]) == 4
                if straddle and pi == last:
                    # finish the p-2 straddle in one go so its PSUM slot is
                    # free before the last pair's own O-tile allocates
                    emit_o_chunk(st[pi - 2], 8, NKT)
                    emit_norm(st[pi - 2])
                elif straddle:
                    emit_o_chunk(st[pi - 2], 8, 12)
                for fn in g3_extra.get(pi, ()):
                    fn()
                if pi == last:
                    emit_o_chunk(st[pi - 1], 8, NKT)
                    emit_norm(st[pi - 1])
                    cur = new_po(pi)
                    emit_o_chunk(cur, 0, 4)
                    emit_o_chunk(cur, 4, 10)
                emit_s_group(qb, hp, pt, 4)
                if straddle and pi != last:
                    emit_o_chunk(st[pi - 2], 12, NKT)
                    emit_norm(st[pi - 2])
                emit_s_group(qb, hp, pt, 5)
                # end slot
                if pi == 0:
                    for tt in range(8):
                        emit_vproj(tt)
                if 1 <= pi < last:
                    emit_o_chunk(new_po(pi - 1), 0, 8)
                for fn in end_extra.get(pi, ()):
                    fn()
                if pi == last:
                    emit_o_chunk(st[pi], 10, NKT)
                    emit_norm(st[pi])

    nc.compile()
    return nc


_NC = None


def _get_nc():
    global _NC
    if _NC is None:
        _NC = build()
    return _NC


def run(query, key, W_query, W_key, W_value, trace=False):
    nc = _get_nc()
    query = np.asarray(query, dtype=np.float32)
    key = np.asarray(key, dtype=np.float32)
    W_query = np.asarray(W_query, dtype=np.float32)
    W_key = np.asarray(W_key, dtype=np.float32)
    W_value = np.asarray(W_value, dtype=np.float32)

    bf = ml_dtypes.bfloat16
    in_maps = []
    for c in range(8):
        n, g = c // 2, c % 2
        cols = slice(g * GC, (g + 1) * GC)
        in_maps.append(
            {
                "qT": np.ascontiguousarray(query[n].T.astype(bf)),
                "kT": np.ascontiguousarray(key[n].T.astype(bf)),
                "wq": np.ascontiguousarray(W_query[:, cols].astype(bf)),
                "wk": np.ascontiguousarray(W_key[:, cols].astype(bf)),
                "wv": np.ascontiguousarray(W_value[:, cols].astype(bf)),
            }
        )
    res = run_bass_kernel_spmd(nc, in_maps, core_ids=list(range(8)), trace=trace)
    out = np.empty((N, T, D), dtype=np.float32)
    for c in range(8):
        n, g = c // 2, c % 2
        out[n, :, g * GC : (g + 1) * GC] = res.results[c]["oT"].T
    return out, res


def kernel(query, key, W_query, W_key, W_value):
    out, _ = run(query, key, W_query, W_key, W_value, trace=False)
    return out


# revision 31
# speedup vs baseline: 1.0150x; 1.0084x over previous
"""Multi-head attention (N=4, T=2048, D=512, H=8, dh=64) on 8 TRN2 NeuronCores.

Sharding: batch N (4) x head-group (2 groups of 4 heads) -> 8 cores.
Each core computes, for its (batch n, head-group g):
  q = query[n] @ Wq[:, 256g:256g+256]   (as qT, [256, 2048])
  k = key[n]   @ Wk[:, ...]             (as kT)
  v = key[n]   @ Wv[:, ...]             (as V tiles [t, dh] with ones column)
  per head h' in 0..3, per q-block of 512:
    ST[k, q] = K-tile matmuls (contraction dh=64, bf16)
    P = exp(ST / sqrt(512))  (ScalarE, multi-bank PSUM read)
    OT[65, 512] += [V | 1]^T @ P  (row 64 = softmax denominators)
    out = OT[0:64] * broadcast(1 / OT[64])
Host reassembles out[n, :, 256g:256g+256] = oT.T.

The kernel is ScalarE(exp)-bound (~127us of Exp activations; fp8 matmuls
were tried and fail the accuracy budget, so everything is bf16 and PE is
~130us, co-critical). The schedule keeps ScalarE busy back-to-back:
  - startup: only K-proj + Q-proj(qb0) gate the first exp; staging DMAs
    are split across the SP and Activation HWDGE queues.
  - all other projections (K dt2=1, V per-head, Q blocks) are emitted in
    bounded chunks at pair ends, sized to the PE slack in each pair's exp
    window (pair 0 has no O-accumulation yet, so it takes the big chunk).
  - the attention loop is software-pipelined: pair p's score/exp phase
    interleaves pair p-1's O-accumulation + normalization; the last pair
    O-accumulates eagerly after each exp group to shorten the tail.
  - projections borrow PSUM transiently: startup/deferred ones cycle
    through the C ring between O-tile lifetimes (emission order keeps the
    ring free of deadlocks).
"""

import math

import ml_dtypes
import numpy as np

import concourse.bass as bass
import concourse.mybir as mybir
import concourse.tile as tile
from concourse import bacc
from concourse.bass_utils import run_bass_kernel_spmd

F32 = mybir.dt.float32
BF16 = mybir.dt.bfloat16
EXP = mybir.ActivationFunctionType.Exp

N, T, D = 4, 2048, 512
HPC, DH = 4, 64          # heads per core, head dim
GC = HPC * DH            # head-group columns (256)
SCALE = 1.0 / math.sqrt(D)
QB = 512                 # q block
NQB = T // QB            # 4
NKT = T // 128           # 16 k tiles
KS = D // 128            # 4 contraction slices for projections

# exp-group pattern per (head, qblock): (pool_key, n_ktiles). Pools A (4 banks)
# and B (2 banks) alternate so TensorE score matmuls overlap ScalarE exp.
GROUPS = (("A", 2), ("B", 2), ("A", 4), ("B", 2), ("A", 4), ("B", 2))


def build():
    nc = bacc.Bacc("TRN2", target_bir_lowering=False, debug=False, num_devices=8)
    qT_in = nc.declare_dram_parameter("qT", [D, T], BF16, isOutput=False)
    kT_in = nc.declare_dram_parameter("kT", [D, T], BF16, isOutput=False)
    wq_in = nc.declare_dram_parameter("wq", [D, GC], BF16, isOutput=False)
    wk_in = nc.declare_dram_parameter("wk", [D, GC], BF16, isOutput=False)
    wv_in = nc.declare_dram_parameter("wv", [D, GC], BF16, isOutput=False)
    oT_out = nc.declare_dram_parameter("oT", [GC, T], F32, isOutput=True)

    with tile.TileContext(nc) as tc:
        with (
            tc.tile_pool(name="stage", bufs=1) as stage,
            tc.tile_pool(name="const", bufs=1) as const,
            tc.tile_pool(name="act", bufs=1) as actp,
            tc.tile_pool(name="pt", bufs=3) as ptp,
            tc.tile_pool(name="small", bufs=4) as small,
            tc.tile_pool(name="psA", bufs=1, space="PSUM") as psA,
            tc.tile_pool(name="psB", bufs=1, space="PSUM") as psB,
            tc.tile_pool(name="psC", bufs=2, space="PSUM") as psC,
        ):
            # ---- input staging on the two HWDGE queues (SP + Activation).
            # gpsimd SWDGE costs ~3.2us of sequencer per strided DMA: not used.
            # Critical path: kin (K-proj) and qin block 0 + wq (Q-proj qb0).
            ws = {}
            for nm in ("wk", "wv", "wq"):
                ws[nm] = const.tile([128, KS, GC], BF16, tag=nm, name=nm)
            kin = stage.tile([128, KS, T], BF16, tag="kin")
            qin = stage.tile([128, KS, T], BF16, tag="qin")

            def dma_in(eng, dst, src_ap):
                eng.dma_start(dst, src_ap)

            kT_r = kT_in.rearrange("(s p) t -> p s t", p=128)
            qT_r = qT_in.rearrange("(s p) t -> p s t", p=128)
            wk_r = wk_in.rearrange("(s p) c -> p s c", p=128)
            wq_r = wq_in.rearrange("(s p) c -> p s c", p=128)
            wv_r = wv_in.rearrange("(s p) c -> p s c", p=128)

            def kin_tb(tb):
                return (kin[:, :, tb * QB : (tb + 1) * QB],
                        kT_r[:, :, tb * QB : (tb + 1) * QB])

            def qin_tb(tb):
                return (qin[:, :, tb * QB : (tb + 1) * QB],
                        qT_r[:, :, tb * QB : (tb + 1) * QB])

            # one serial ~360GB/s DMA pipe: global order matters, queues only
            # pipeline the config step. Order: wk, wq, kin0, qin0, wv, kin1-3,
            # then the remaining q blocks.
            nc.sync.dma_start(ws["wk"][:], wk_r)
            nc.scalar.dma_start(ws["wq"][:], wq_r)
            nc.sync.dma_start(*qin_tb(0))
            nc.scalar.dma_start(*kin_tb(0))
            nc.sync.dma_start(*kin_tb(1))
            nc.scalar.dma_start(*kin_tb(2))
            nc.sync.dma_start(*kin_tb(3))
            nc.scalar.dma_start(ws["wv"][:], wv_r)
            nc.sync.dma_start(*qin_tb(1))
            nc.scalar.dma_start(*qin_tb(2))
            nc.sync.dma_start(*qin_tb(3))

            kT_att = [
                actp.tile([128, T], BF16, tag=f"ka{d}", name=f"ka{d}")
                for d in range(2)
            ]
            qT_att = [
                actp.tile([128, T], BF16, tag=f"qa{d}", name=f"qa{d}")
                for d in range(2)
            ]

            def emit_kproj_tb(dt2, tb, copy_eng="dve"):
                ps = psC.tile([128, QB], F32, tag="C", name="kproj_ps")
                for s in range(KS):
                    nc.tensor.matmul(
                        ps[:],
                        ws["wk"][:, s, dt2 * 128 : (dt2 + 1) * 128],
                        kin[:, s, tb * QB : (tb + 1) * QB],
                        start=(s == 0),
                        stop=(s == KS - 1),
                    )
                dst = kT_att[dt2][:, tb * QB : (tb + 1) * QB]
                if copy_eng == "act":
                    # ScalarE is idle pre-attention: Copy activation offloads
                    # the startup-critical PSUM->SBUF move from DVE
                    nc.scalar.copy(dst, ps[:])
                else:
                    nc.vector.tensor_copy(dst, ps[:])

            def emit_qproj(qb, dt2, copy_eng="dve"):
                ps = psC.tile([128, QB], F32, tag="C", name="qproj_ps")
                for s in range(KS):
                    nc.tensor.matmul(
                        ps[:],
                        ws["wq"][:, s, dt2 * 128 : (dt2 + 1) * 128],
                        qin[:, s, qb * QB : (qb + 1) * QB],
                        start=(s == 0),
                        stop=(s == KS - 1),
                    )
                dst = qT_att[dt2][:, qb * QB : (qb + 1) * QB]
                if copy_eng == "act":
                    nc.scalar.copy(dst, ps[:])
                else:
                    nc.vector.tensor_copy(dst, ps[:])

            # ---- V tiles [128, kt, head, 65] with ones column ----
            vp = const.tile([128, NKT, HPC, DH + 1], BF16, tag="vp")
            ones_f32 = const.tile([128, NKT * HPC], F32, tag="ones")
            nc.vector.memset(ones_f32[:], 1.0)
            nc.vector.tensor_copy(
                vp[:, :, :, DH : DH + 1],
                ones_f32[:].rearrange("p (a b) -> p a b", b=HPC).unsqueeze(3),
            )

            def emit_vproj(tt):
                # all 4 heads' V for one k-tile: out [128 kpos, 256]
                ps = psC.tile([128, QB], F32, tag="C", name="vproj_ps")
                for s in range(KS):
                    nc.tensor.matmul(
                        ps[:, 0:GC],
                        kin[:, s, tt * 128 : (tt + 1) * 128],
                        ws["wv"][:, s, :],
                        start=(s == 0),
                        stop=(s == KS - 1),
                    )
                nc.vector.tensor_copy(
                    vp[:, tt, :, 0:DH],
                    ps[:, 0:GC].rearrange("p (h d) -> p h d", d=DH),
                )

            # ---- attention pieces ----
            def emit_s_group(qb, hp, pt, gi):
                pool_key, nkt = GROUPS[gi]
                kt0 = sum(n for _, n in GROUPS[:gi])
                tile2, base = hp // 2, DH * (hp % 2)
                q_src = qT_att[tile2][base : base + DH, qb * QB : (qb + 1) * QB]
                pool = psA if pool_key == "A" else psB
                width = 2048 if pool_key == "A" else 1024
                ps = pool.tile([128, width], F32, tag=pool_key, name="s_ps")
                for l in range(nkt):
                    kt = kt0 + l
                    nc.tensor.matmul(
                        ps[:, l * QB : (l + 1) * QB],
                        kT_att[tile2][base : base + DH, kt * 128 : (kt + 1) * 128],
                        q_src,
                        start=True,
                        stop=True,
                    )
                nc.scalar.activation(
                    pt[:, kt0 * QB : (kt0 + nkt) * QB],
                    ps[:, : nkt * QB],
                    EXP,
                    scale=SCALE,
                )

            def emit_o_chunk(st, kt_lo, kt_hi):
                qb, hp, pt, po = st
                for kt in range(kt_lo, kt_hi):
                    nc.tensor.matmul(
                        po[0 : DH + 1],
                        vp[:, kt, hp, :],
                        pt[:, kt * QB : (kt + 1) * QB],
                        start=(kt == 0),
                        stop=(kt == NKT - 1),
                    )

            def emit_norm(st, halves=1):
                # halves=2 pipelines the copy/recip/broadcast/mul/dma chain
                # across DVE/Pool/SP for the tail-critical last pair
                qb, hp, pt, po = st
                hw = QB // halves
                for h in range(halves):
                    cs = slice(h * hw, (h + 1) * hw)
                    sums = small.tile([1, hw], F32, tag="sums", name="sums")
                    nc.vector.tensor_copy(sums[:], po[DH : DH + 1, cs])
                    rec = small.tile([1, hw], F32, tag="rec", name="rec")
                    nc.vector.reciprocal_approx_fast(rec[:], sums[:])
                    bc = small.tile([DH, hw], F32, tag="bc", name="bc")
                    nc.gpsimd.partition_broadcast(bc[:], rec[:])
                    ot = small.tile([DH, hw], F32, tag="ot", name="ot")
                    nc.vector.tensor_mul(ot[:], po[0:DH, cs], bc[:])
                    nc.sync.dma_start(
                        oT_out[
                            hp * DH : (hp + 1) * DH,
                            qb * QB + h * hw : qb * QB + (h + 1) * hw,
                        ],
                        ot[:],
                    )

            # ---- schedule ----
            # Pair order is head-pair-major: all q blocks for heads 0-1 first,
            # then heads 2-3 -- this pushes the K-proj dt2=1 deadline from
            # pair 2 out to pair 8 so it amortizes into exp windows.
            #
            # AV(p) is straddled: chunk1 (kt 0-8) at the end of pair p+1,
            # chunk2 (kt 8-16) + norm after group 3 of pair p+2. That caps
            # per-slot PE bursts at ~1.7us, leaving slot room for deferred
            # projections (V-proj, K dt2=1, Q blocks) without stalling exp.
            pairs = [
                (qb, hp)
                for dt2 in (0, 1)
                for qb in range(NQB)
                for hp in (2 * dt2, 2 * dt2 + 1)
            ]
            last = len(pairs) - 1
            st = {}

            def new_po(pi):
                po = psC.tile([128, QB], F32, tag="C", name="po")
                st[pi] = (*st[pi], po)
                return st[pi]

            g1_slot = {
                1: [lambda: [emit_vproj(tt) for tt in range(8, 12)]],
                2: [lambda: [emit_vproj(tt) for tt in range(12, 16)]],
                4: [lambda: emit_kproj_tb(1, 0)],
                5: [lambda: emit_kproj_tb(1, 1)],
                6: [lambda: emit_kproj_tb(1, 2)],
                7: [lambda: emit_kproj_tb(1, 3)],
            }
            g3_extra = {}
            end_extra = {
                1: [lambda: emit_qproj(1, 0)],
                3: [lambda: emit_qproj(2, 0)],
                5: [lambda: emit_qproj(3, 0)],
                6: [lambda: emit_qproj(0, 1)],
                8: [lambda: emit_qproj(1, 1)],
                10: [lambda: emit_qproj(2, 1)],
                12: [lambda: emit_qproj(3, 1)],
            }

            # PE warmup: ~12 junk matmuls with no data deps ride out the
            # p-state ramp while the first DMAs are in flight. They write the
            # first psA ring slot, which pair 0's group 0 then overwrites
            # (start=True resets the accumulation region).
            dummy = const.tile([64, 128], BF16, tag="dummy")
            nc.gpsimd.memset(dummy[:], 0.0)
            wps = psA.tile([128, 2048], F32, tag="A", name="warm_ps")
            for _ in range(32):
                nc.tensor.matmul(
                    wps[:, 0:128],
                    dummy[:],
                    dummy[:],
                    start=True,
                    stop=True,
                )

            # startup: Q-proj(qb0) first (its q block is first on the wire),
            # then K-proj dt2=0 per arriving kin block, interleaved with
            # pair 0's score groups below.
            emit_qproj(0, 0, copy_eng="act")
            emit_kproj_tb(0, 0, copy_eng="act")

            for pi, (qb, hp) in enumerate(pairs):
                pt = ptp.tile([128, NKT * QB], BF16, tag="pt", name="pt")
                st[pi] = (qb, hp, pt)
                emit_s_group(qb, hp, pt, 0)
                emit_s_group(qb, hp, pt, 1)
                # g1 slot
                for fn in g1_slot.get(pi, ()):
                    fn()
                if pi == 0:
                    emit_kproj_tb(0, 1)
                if pi == last:
                    emit_o_chunk(new_po(pi - 1), 0, 8)
                emit_s_group(qb, hp, pt, 2)
                if pi == 0:
                    emit_kproj_tb(0, 2)
                emit_s_group(qb, hp, pt, 3)
                # g3 slot
                if pi == 0:
                    emit_kproj_tb(0, 3)
                straddle = pi >= 2 and pi - 2 in st and len(st[pi - 2]) == 4
                if straddle and pi == last:
                    # finish the p-2 straddle in one go so its PSUM slot is
                    # free before the last pair's own O-tile allocates
                    emit_o_chunk(st[pi - 2], 8, NKT)
                    emit_norm(st[pi - 2])
                elif straddle:
                    emit_o_chunk(st[pi - 2], 8, 12)
                for fn in g3_extra.get(pi, ()):
                    fn()
                if pi == last:
                    emit_o_chunk(st[pi - 1], 8, NKT)
                    emit_norm(st[pi - 1])
                    cur = new_po(pi)
                    emit_o_chunk(cur, 0, 4)
                emit_s_group(qb, hp, pt, 4)
                if straddle and pi != last:
                    emit_o_chunk(st[pi - 2], 12, NKT)
                    emit_norm(st[pi - 2])
                if pi == last:
                    emit_o_chunk(st[pi], 4, 10)
                if pi == last:
                    # kt 10-13 are ready after group 4's exp; only kt 14-15
                    # remain after the final act
                    emit_o_chunk(st[pi], 10, 14)
                emit_s_group(qb, hp, pt, 5)
                # end slot
                if pi == 0:
                    for tt in range(8):
                        emit_vproj(tt)
                if 1 <= pi < last:
                    emit_o_chunk(new_po(pi - 1), 0, 8)
                for fn in end_extra.get(pi, ()):
                    fn()
                if pi == last:
                    emit_o_chunk(st[pi], 14, NKT)
                    emit_norm(st[pi])

    nc.compile()
    return nc


_NC = None


def _get_nc():
    global _NC
    if _NC is None:
        _NC = build()
    return _NC


def run(query, key, W_query, W_key, W_value, trace=False):
    nc = _get_nc()
    query = np.asarray(query, dtype=np.float32)
    key = np.asarray(key, dtype=np.float32)
    W_query = np.asarray(W_query, dtype=np.float32)
    W_key = np.asarray(W_key, dtype=np.float32)
    W_value = np.asarray(W_value, dtype=np.float32)

    bf = ml_dtypes.bfloat16
    in_maps = []
    for c in range(8):
        n, g = c // 2, c % 2
        cols = slice(g * GC, (g + 1) * GC)
        in_maps.append(
            {
                "qT": np.ascontiguousarray(query[n].T.astype(bf)),
                "kT": np.ascontiguousarray(key[n].T.astype(bf)),
                "wq": np.ascontiguousarray(W_query[:, cols].astype(bf)),
                "wk": np.ascontiguousarray(W_key[:, cols].astype(bf)),
                "wv": np.ascontiguousarray(W_value[:, cols].astype(bf)),
            }
        )
    res = run_bass_kernel_spmd(nc, in_maps, core_ids=list(range(8)), trace=trace)
    out = np.empty((N, T, D), dtype=np.float32)
    for c in range(8):
        n, g = c // 2, c % 2
        out[n, :, g * GC : (g + 1) * GC] = res.results[c]["oT"].T
    return out, res


def kernel(query, key, W_query, W_key, W_value):
    out, _ = run(query, key, W_query, W_key, W_value, trace=False)
    return out
